# revision 5
# baseline (speedup 1.0000x reference)
"""Soft decision-tree (MoE-routing style) model on 8 Trainium2 NeuronCores.

Computation (see reference):
    d      = sigmoid(x @ W^T)                  x:[B,1024]  W:[1023,1024]
    probs  = level-by-level path products       -> [B, 1024] leaf probs
    out    = softmax(probs @ L, axis=1)         L:[1024,1024]

Strategy (per core, data-parallel over batch):
  * Contraction-on-partitions layout everywhere -> no transposes on device:
      GEMM1: z[slot, b]   = Wp^T-chunks (lhsT) x x^T-chunks (rhs)
      GEMM2: logit[b, o]  = P10-chunks  (lhsT) x L-chunks   (rhs)
  * Host pre-permutes weights:
      - node dim padded 1023 -> 1024 "slots", level l at [2^l, 2^(l+1)),
        little-endian order within the level (tree recursion is pure
        concat, never interleave).
      - leaf predictions permuted by 10-bit bit-reversal to match.
      - every device input is laid out so each DMA is a dense 2D
        transfer (128 partitions x 2KB+ contiguous rows) -- the strided
        gathers were costing 3x in descriptor pressure + HBM efficiency.
  * GEMM1 runs a single fp32r pass (x, W rounded to fp32r's 12-bit
    mantissa, RNE). The rounding puts ~5e-3 absolute error on z whose
    std is ~32; through sigmoid/softmax that lands at ~3e-3 relative on
    the output -- well inside the 2e-2 gate. (Each matmul instruction
    costs the same ~230 ns on HW whether or not fp8 DoubleRow packs 2x
    the contraction, so correction passes cost full PE time and were
    dropped.)
  * Tree levels 0-6 are evaluated in log-space with one fp32 PE matmul
    group against a constant 0/1 selection matrix (M7) so every
    vector-engine op is full-width and partition-aligned. The M7 matmul
    + exp are emitted BEFORE the previous block's GEMM2 so the 7us DVE
    tree chain overlaps GEMM2 instead of serializing after it.
  * Levels 7-9 are plain full-width f32 mul/sub; the last level writes
    float32r (rounded) because GEMM2 consumes it as the stationary operand.
  * ACT ops are clustered by function per block (sigmoid x9, then
    ln/exp which share one activation table) to avoid table-reload thrash.
  * Matmul free size is capped at 512 fp32 (one PSUM bank) by the ISA,
    so GEMM2 splits the 1024-wide output row into two 512 halves.
"""

import numpy as np

import concourse.bacc as bacc
import concourse.bass as bass
import concourse.mybir as mybir
import concourse.tile as tile
from concourse.bass_utils import run_bass_kernel_spmd

AF = mybir.ActivationFunctionType
f32 = mybir.dt.float32
f32r = mybir.dt.float32r

MAX_DEPTH = 10
B = 32768
F = 1024
NOUT = 1024
NLEAF = 1024
NCORES = 8
BL = B // NCORES          # rows per core
BLOCK = 512               # batch columns processed per block
NBLOCKS = BL // BLOCK


def _bitrev(i: int, bits: int) -> int:
    r = 0
    for b in range(bits):
        r = (r << 1) | ((i >> b) & 1)
    return r


def _round_f32r(a: np.ndarray) -> np.ndarray:
    """Round fp32 to fp32r (1s/8e/11m, value held in the top 20 bits), RNE."""
    u = np.ascontiguousarray(a, dtype=np.float32).view(np.uint32)
    lo = u & np.uint32(0xFFF)
    base = u & np.uint32(0xFFFFF000)
    rnd = (lo > 0x800) | ((lo == 0x800) & (((u >> np.uint32(12)) & np.uint32(1)) == 1))
    out = base + (rnd.astype(np.uint32) << np.uint32(12))
    return out.view(np.float32)


def _host_prep(feature_thresholds: np.ndarray, leaf_predictions: np.ndarray):
    """Build the permuted/padded constant tensors (blocked DMA layouts)."""
    ft = np.asarray(feature_thresholds, dtype=np.float32)
    lp = np.asarray(leaf_predictions, dtype=np.float32)

    # Padded node slots: level l occupies [2^l, 2^(l+1)), little-endian order
    # within the level: slot 2^l + j holds BFS node (2^l - 1) + bitrev_l(j).
    wp = np.zeros((1024, F), dtype=np.float32)
    for lvl in range(MAX_DEPTH):
        n = 1 << lvl
        src = np.fromiter(
            ((n - 1) + _bitrev(j, lvl) for j in range(n)), dtype=np.int64, count=n
        )
        wp[n : 2 * n] = ft[src]
    wt = _round_f32r(wp.T)  # [F, 1024 slots]
    # -> [nch, c, p, n]: chunk (nch, c) is a dense [128, 128] 2D block.
    wt_b = np.ascontiguousarray(
        wt.reshape(8, 128, 8, 128).transpose(2, 0, 1, 3)
    )

    # Leaf predictions in little-endian leaf order, [lc, p, o] blocked.
    perm = np.fromiter(
        (_bitrev(i, MAX_DEPTH) for i in range(NLEAF)), dtype=np.int64, count=NLEAF
    )
    lp_b = np.ascontiguousarray(_round_f32r(lp[perm]).reshape(8, 128, NOUT))

    # M7 selection matrix: logP7[j] = sum over levels 0..6 of ln(d or 1-d).
    # Rows 0..127   multiply ln(sigmoid(z))  of slot s.
    # Rows 128..255 multiply ln(sigmoid(-z)) of slot s-128.
    m7 = np.zeros((256, 128), dtype=np.float32)
    for j in range(128):
        for lvl in range(7):
            slot = (1 << lvl) + (j & ((1 << lvl) - 1))
            bit = (j >> lvl) & 1
            m7[slot + 128 * bit, j] = 1.0
    return wt_b, lp_b, m7.reshape(2, 128, 128)  # m7 is 0/1: exact in fp32r


def _build_program(n_blocks: int = NBLOCKS, block: int = BLOCK) -> bass.Bass:
    nc = bacc.Bacc()
    nb = n_blocks * block
    # xt blocked [bi, c, p, b]: each (bi, c) is a dense [128, block] 2D chunk.
    xt = nc.dram_tensor("xt", [n_blocks, 8, 128, block], f32r, kind="ExternalInput")
    wt = nc.dram_tensor("wt", [8, 8, 128, 128], f32r, kind="ExternalInput")
    lp = nc.dram_tensor("lp", [8, 128, NOUT], f32r, kind="ExternalInput")
    m7 = nc.dram_tensor("m7", [2, 128, 128], f32r, kind="ExternalInput")
    out = nc.dram_tensor("out", [nb, NOUT], f32, kind="ExternalOutput")

    with tile.TileContext(nc) as tc:
        with (
            tc.tile_pool(name="consts", bufs=1) as consts,
            tc.tile_pool(name="xtp", bufs=2) as xtp,
            tc.tile_pool(name="dp", bufs=2) as dp,
            tc.tile_pool(name="sgp", bufs=1) as sgp,
            tc.tile_pool(name="lnfull", bufs=1) as lnfull,
            tc.tile_pool(name="tree89", bufs=1) as tree89,
            tc.tile_pool(name="p10pool", bufs=2) as p10pool,
            tc.tile_pool(name="outp", bufs=2) as outp,
            tc.tile_pool(name="smalls", bufs=2) as smalls,
            tc.tile_pool(name="zps", bufs=2, space="PSUM") as zps,
            tc.tile_pool(name="p7ps", bufs=2, space="PSUM") as p7ps,
            tc.tile_pool(name="gps", bufs=2, space="PSUM") as gps,
        ):
            wt_sb_n = []
            for k in range(8):
                t = consts.tile([128, 8, 128], f32r, tag=f"wt{k}")
                wt_sb_n.append(t)
            nc.sync.dma_start(out=wt_sb_n[0], in_=wt[0].rearrange("c p n -> p c n"))

            def emit_g2_softmax(st):
                bs_prev, p10_prev = st
                for sb in range(block // 128):
                    g = gps.tile([128, 1024], f32, tag="g")
                    for h in range(2):
                        for lc in range(8):
                            nc.tensor.matmul(
                                g[:, h * 512 : (h + 1) * 512],
                                lhsT=p10_prev[lc][:, sb * 128 : (sb + 1) * 128],
                                rhs=lp_sb[:, lc, h * 512 : (h + 1) * 512],
                                start=(lc == 0),
                                stop=(lc == 7),
                            )
                    out_t = outp.tile([128, NOUT], f32, tag="out")
                    stot = smalls.tile([128, 1], f32, tag="stot")
                    # |logits| <= max|leaf_pred| (convex combination): exp is
                    # overflow-safe without a max-subtraction pass.
                    nc.scalar.activation(
                        out=out_t, in_=g, func=AF.Exp, accum_out=stot
                    )
                    rcp = smalls.tile([128, 1], f32, tag="rcp")
                    nc.vector.reciprocal(rcp, stot)
                    nc.vector.tensor_scalar_mul(out_t, out_t, rcp)
                    nc.sync.dma_start(
                        out=out[bs_prev + sb * 128 : bs_prev + (sb + 1) * 128, :],
                        in_=out_t,
                    )

            def load_block(bi):
                xt_sb = xtp.tile([128, 8, block], f32r, tag="xt")
                nc.sync.dma_start(out=xt_sb, in_=xt[bi].rearrange("c p b -> p c b"))
                return xt_sb

            pending = None
            xt_sb = load_block(0)
            for k in range(1, 8):
                nc.sync.dma_start(
                    out=wt_sb_n[k], in_=wt[k].rearrange("c p n -> p c n")
                )
            lp_sb = consts.tile([128, 8, NOUT], f32r)
            nc.gpsimd.dma_start(out=lp_sb, in_=lp.rearrange("c p o -> p c o"))
            m7_sb = consts.tile([128, 2, 128], f32r)
            nc.gpsimd.dma_start(out=m7_sb, in_=m7.rearrange("c p j -> p c j"))
            ln_eps = consts.tile([128, 1], f32)
            nc.vector.memset(ln_eps, 1e-37)
            for bi in range(n_blocks):
                # ---- GEMM1: z[slot, b] = one fp32r pass ----
                zpsums = {}
                for nch in range(8):
                    zp = zps.tile([128, block], f32, tag="z")
                    for fc in range(8):
                        nc.tensor.matmul(
                            zp,
                            lhsT=wt_sb_n[nch][:, fc, :],
                            rhs=xt_sb[:, fc, :],
                            start=(fc == 0), stop=(fc == 7),
                        )
                    zpsums[nch] = zp

                # ---- ACT phase 1: all sigmoids (one table) ----
                sg_pos = sgp.tile([128, block], f32, tag="sgpos")
                nc.scalar.activation(out=sg_pos, in_=zpsums[0], func=AF.Sigmoid)
                sg_neg = sgp.tile([128, block], f32, tag="sgneg")
                nc.scalar.activation(
                    out=sg_neg, in_=zpsums[0], func=AF.Sigmoid, scale=-1.0
                )
                dcs = {}
                for nch in range(1, 8):
                    d = dp.tile([128, block], f32, tag=f"dc{nch}")
                    nc.scalar.activation(out=d, in_=zpsums[nch], func=AF.Sigmoid)
                    dcs[nch] = d

                if bi + 1 < n_blocks:
                    next_xt = load_block(bi + 1)
                else:
                    next_xt = None

                # ---- ACT phase 2: both lns right after the sigmoids so the
                # ACT table sequence is SIG -> LN -> EXP once per block ----
                lnf_p = lnfull.tile([128, block], f32r, tag="lnfp")
                nc.scalar.activation(out=lnf_p, in_=sg_pos, func=AF.Ln, bias=ln_eps)
                lnf_n = sgp.tile([128, block], f32r, tag="sgpos")
                nc.scalar.activation(out=lnf_n, in_=sg_neg, func=AF.Ln, bias=ln_eps)

                # ---- levels 0-6 in log space on the PE (fp32r: the ln
                # rounding is ~1.2e-4 relative, well inside the error floor).
                # Emitted BEFORE the previous block's GEMM2 so the DVE tree
                # below overlaps that GEMM2 instead of serializing after. ----
                lp7 = p7ps.tile([128, block], f32, tag="logp7")
                nc.tensor.matmul(lp7, lhsT=m7_sb[:, 0, :], rhs=lnf_p, start=True, stop=False)
                nc.tensor.matmul(lp7, lhsT=m7_sb[:, 1, :], rhs=lnf_n, start=False, stop=True)
                p7 = tree89.tile([128, block], f32, tag="p9_0")
                nc.scalar.activation(out=p7, in_=lp7, func=AF.Exp)

                # ---- levels 7-9, all full-width partition-aligned f32 ----
                p8a = tree89.tile([128, block], f32, tag="p8a")
                nc.vector.tensor_mul(p8a, p7, dcs[1])
                p8b = tree89.tile([128, block], f32, tag="p8b")
                nc.vector.tensor_sub(p8b, p7, p8a)

                p9 = []
                t = tree89.tile([128, block], f32, tag="p9_0")
                nc.vector.tensor_mul(t, p8a, dcs[2])
                p9.append(t)
                t = tree89.tile([128, block], f32, tag="p9_1")
                nc.vector.tensor_mul(t, p8b, dcs[3])
                p9.append(t)
                # in-place: p8a/p8b become p9_2/p9_3
                nc.vector.tensor_sub(p8a, p8a, p9[0])
                p9.append(p8a)
                nc.vector.tensor_sub(p8b, p8b, p9[1])
                p9.append(p8b)

                # level 9 writes fp32r (GEMM2 stationary operand)
                p10 = [None] * 8
                for k in range(4):
                    t = p10pool.tile([128, block], f32r, tag=f"p10_{k}")
                    nc.vector.tensor_mul(t, p9[k], dcs[4 + k])
                    p10[k] = t
                for k in range(4):
                    t = p10pool.tile([128, block], f32r, tag=f"p10_{4 + k}")
                    nc.vector.tensor_sub(t, p9[k], p10[k].bitcast(f32))
                    p10[4 + k] = t

                # ---- GEMM2 + softmax of the previous block (PE fills the
                # gap while ACT/DVE work through this block's tree) ----
                if pending is not None:
                    emit_g2_softmax(pending)

                pending = (bi * block, p10)
                if next_xt is not None:
                    xt_sb = next_xt

            emit_g2_softmax(pending)
    nc.finalize()
    return nc


_PROGRAM_CACHE: dict = {}


def _get_program(n_blocks: int = NBLOCKS, block: int = BLOCK) -> bass.Bass:
    key = (n_blocks, block)
    if key not in _PROGRAM_CACHE:
        _PROGRAM_CACHE[key] = _build_program(n_blocks, block)
    return _PROGRAM_CACHE[key]


def kernel(x, feature_thresholds, leaf_predictions, _trace=False):
    x = np.asarray(x, dtype=np.float32)
    wt_b, lp_b, m7_b = _host_prep(feature_thresholds, leaf_predictions)
    xt_r = _round_f32r(x.T)  # [F, B]

    nc = _get_program()
    in_maps = []
    for c in range(NCORES):
        xt_core = xt_r[:, c * BL : (c + 1) * BL]
        # [bi, c, p, b]: each (bi, c) chunk dense [128, block]
        xt_blk = np.ascontiguousarray(
            xt_core.reshape(8, 128, NBLOCKS, BLOCK).transpose(2, 0, 1, 3)
        )
        in_maps.append({"xt": xt_blk, "wt": wt_b, "lp": lp_b, "m7": m7_b})

    res = run_bass_kernel_spmd(nc, in_maps, core_ids=list(range(NCORES)), trace=_trace)
    out = np.concatenate([res.results[c]["out"] for c in range(NCORES)], axis=0)
    if _trace:
        return out, res
    return out
